# revision 55
# baseline (speedup 1.0000x reference)
"""Bending-energy loss kernel for Trainium2 (8 NeuronCores, Bass/Tile).

Input: ddf (4, 160, 160, 160, 3) fp32. Output: (4,) fp32 per-batch mean
bending energy.

Sharding: core = (batch, x-half). Per core the volume is processed in a
[y-partition, x-block, z*c] layout:
  - DVE computes the pure-x second-difference fields (xx, most of xz) as
    shifted tensor_subs in bf16 (free-dim shifts only); Pool (GPSIMD)
    computes zz and the tail of xz.
  - TensorE applies y-direction stencils as banded-matrix matmuls
    (yy, xy, yz), and squares+reduces the SBUF-resident fields via
    accumulating Gram matmuls (diag of lhsT.T@lhsT accumulated in PSUM).
  - ScalarE squares the PSUM-resident y-fields into SBUF bf16 tiles
    (no accum_out - cheaper); TensorE reduces those with ones-vector
    matmuls into dedicated PSUM accumulator columns.
  - A tunable subset of main-tile xy blocks ("u-path") instead uses two
    partition-shifted DMA copies of gx + a DVE sub + a PE gram, moving
    work from ACT to the underused DMA/DVE.
y=160 does not fit 128 partitions: a main tile y[0,128) plus a 36-row
strip y[124,160) packed host-side as (3 x-groups x 36 rows) = 108
partitions. Double-counted / out-of-window y planes are subtracted on
the host (8 of 160 planes per batch, O(1%) of the data).
"""

import numpy as np
import ml_dtypes

B = 4
D = 160
C = 3
ZC = D * C          # 480
NCORES = 8

XIN = 82            # x planes per core (incl. +-2 halo)
NXOUT = 78          # energy x-blocks per core (local x in [2, 80))
XO = 12             # out-blocks per main-tile macro group
YS0, YSN = 124, 36  # strip y range [124, 160)
SGRP = 3            # strip x groups
SOUT = 26           # out-blocks per strip group
SIN = 30            # in-blocks per strip group
SSUB = 9            # strip sub-group out blocks
PK = SGRP * YSN     # 108 packed strip partitions
MA = 124            # main-tile PE output rows (y-out [2,126))
MS = SGRP * 32      # strip PE output rows (y-out [126,158) per group)

# z energy window [2,158): xx crop cols [6,474), xz [3,471), zz tile already 468
ZW = 468
CHUNK = 3           # blocks per PSUM tile / ACT square
PCHUNK = 4          # blocks per Pool zz sub-chunk
PP_BUFS = 2         # PSUM produce-tile double buffering
FILL_FLOOR = 500.0  # min ns of deferred PE work issued per chunk
ONES_AGE = 3        # chunks of aging before a ones-reduction is eligible
U_OF, U_MOD = 1, 2  # route U_OF of every U_MOD main xy blocks via DMA-shift
U_EXTRA = 0         # additionally route every Nth block (spread densifier)
EARLY_U = -1        # early groups (gidx<=2): -1 normal, 0 no u-path, 1 all u-path
PZ_EVERY = 4        # every Nth main xz block produced on Pool instead of DVE
FLD_BUFS = 2
F_BUFS = 3
G_BUFS = 2

_cache = {}


def _tile_groups():
    """Main-tile macro groups: (in_start, n_in, n_out). Out-block k of a
    group covers local x = in_start + 2 + k."""
    groups = [(0, 7, 3), (3, 10, 6), (9, 13, 9)]
    done = 18
    while done < NXOUT:
        nout = min(XO, NXOUT - done)
        groups.append((done, nout + 4, nout))
        done += nout
    return groups


def _strip_subs():
    """Strip sub-groups within the packed [108, SIN, 480] input:
    (in_start, n_in, n_out)."""
    subs = []
    done = 0
    while done < SOUT:
        nout = min(SSUB, SOUT - done)
        if SOUT - done - nout == 0 and nout > 4:
            # split the final sub in two so the kernel tail is short
            a = nout // 2
            subs.append((done, a + 4, a))
            subs.append((done + a, nout - a + 4, nout - a))
        else:
            subs.append((done, nout + 4, nout))
        done += nout
    return subs


def _stencil_mats():
    xb_a = np.zeros((128, MA), np.float32)
    sa_a = np.zeros((128, MA), np.float32)
    for m in range(MA):
        xb_a[m, m] = 1.0
        xb_a[m + 2, m] = -2.0
        xb_a[m + 4, m] = 1.0
        sa_a[m + 1, m] = -1.0
        sa_a[m + 3, m] = 1.0
    xb_s = np.zeros((PK, MS), np.float32)
    sa_s = np.zeros((PK, MS), np.float32)
    for g in range(SGRP):
        for j in range(32):
            m = 32 * g + j
            r = YSN * g + j
            xb_s[r, m] = 1.0
            xb_s[r + 2, m] = -2.0
            xb_s[r + 4, m] = 1.0
            sa_s[r + 1, m] = -1.0
            sa_s[r + 3, m] = 1.0
    bf = ml_dtypes.bfloat16
    return (xb_a.astype(bf), sa_a.astype(bf), xb_s.astype(bf), sa_s.astype(bf))


def _patch_tile(tile_mod, bass_rust, mybir):
    """Walrus on this toolchain accepts at most ONE sync wait per
    instruction. (a) emit the TileContext exit drain as a chain of
    single-wait drains; (b) post-pass splitting any multi-wait
    instruction with preceding single-wait NoOps."""
    if getattr(tile_mod.TileContext, "_bending_patched", False):
        return

    def _drain_and_barrier_chunked(self, tick_clock, wait_clock):
        nc = self.nc
        gc = tick_clock.global_clock
        items = gc.items() if hasattr(gc, "items") else [(None, gc)]
        reqs = []
        for scope, vclock in items:
            for proc in range(len(vclock)):
                t = vclock[proc]
                if t > 0:
                    reqs.append((scope, proc, t))
        for scope, proc, t in reqs:
            sc = bass_rust.ScopedClock()
            sc.require_at_least(scope, proc, t)
            drain_inst = nc.sync.drain()
            wait_clock.add_sem_waits(drain_inst.ins, sc)
        if not reqs:
            nc.sync.drain()
        nc.all_engine_barrier()
        assert self.sems is not None
        popped = nc._tile_sem_poison_stack.pop()
        assert popped is self._sem_poison
        nc.clear_and_free_semaphores(list(self.sems.allocated().values()))
        nc.all_engine_barrier()

    tile_mod.TileContext._drain_and_barrier = _drain_and_barrier_chunked
    tile_mod.TileContext._bending_patched = True


_nop_counter = [0]


def _split_multi_waits(nc, mybir):
    for bb_name, bb_entry in list(nc.bb_map.items()):
        bb = bb_entry.bb if hasattr(bb_entry, "bb") else bb_entry
        insts = list(bb.instructions)
        new_insts = []
        changed = False
        for inst in insts:
            si = inst.sync_info
            if si is not None and si.on_wait is not None and len(si.on_wait) > 1:
                waits = list(si.on_wait)
                for w in waits[:-1]:
                    _nop_counter[0] += 1
                    nop = mybir.InstNoOp(
                        name=f"I-waitsplit-{_nop_counter[0]}",
                        engine=inst.engine,
                        ins=[],
                        outs=[],
                    )
                    nop.sync_info = mybir.SyncInfo(on_wait=[w], on_update=[])
                    new_insts.append(nop)
                inst.sync_info = mybir.SyncInfo(
                    on_wait=[waits[-1]], on_update=si.on_update
                )
                changed = True
            new_insts.append(inst)
        if changed:
            bb.instructions = new_insts


# PSUM accumulator bank layout (all in one 2KB bank, f32):
#   cols [0,128)   g1: gram of weight-1 fields (xx, zz, u-path yy)
#   cols [128,256) g2: gram of weight-2 fields (xz, u-path xy/yz)
#   col  256       s1: ones-matmul reduction of weight-1 ACT squares (yy)
#   col  257       s2: ones-matmul reduction of weight-2 ACT squares (xy, yz)
GCOLS = 260


def _build_program():
    import bass_rust
    import concourse.bass as bass
    import concourse.tile as tile
    import concourse.mybir as mybir

    _patch_tile(tile, bass_rust, mybir)

    bf16 = mybir.dt.bfloat16
    f32 = mybir.dt.float32
    SQ = mybir.ActivationFunctionType.Square

    groups = _tile_groups()
    subs = _strip_subs()

    nc = bass.Bass()
    fy_d = nc.declare_dram_parameter("fy", [D, XIN, ZC], bf16, isOutput=False)
    fs_d = nc.declare_dram_parameter("fs", [PK, SIN, ZC], bf16, isOutput=False)
    xba_d = nc.declare_dram_parameter("xba", [128, MA], bf16, isOutput=False)
    saa_d = nc.declare_dram_parameter("saa", [128, MA], bf16, isOutput=False)
    xbs_d = nc.declare_dram_parameter("xbs", [PK, MS], bf16, isOutput=False)
    sas_d = nc.declare_dram_parameter("sas", [PK, MS], bf16, isOutput=False)
    ones_d = nc.declare_dram_parameter("ones", [128, 1], bf16, isOutput=False)
    g_d = nc.declare_dram_parameter("g", [128, GCOLS], f32, isOutput=True)

    with tile.TileContext(nc) as tc:
        with (
            tc.tile_pool(name="wpool", bufs=1) as wpool,
            tc.tile_pool(name="fpool", bufs=F_BUFS) as fpool,
            tc.tile_pool(name="gxpool", bufs=2) as gxpool,
            tc.tile_pool(name="gzpool", bufs=G_BUFS) as gzpool,
            tc.tile_pool(name="fldpool", bufs=FLD_BUFS) as fldpool,
            tc.tile_pool(name="ushift", bufs=9) as ushift,
            tc.tile_pool(name="uxy", bufs=6) as uxy,
            tc.tile_pool(name="sqpool", bufs=4) as sqpool,
            tc.tile_pool(name="accpool", bufs=1) as accpool,
            tc.tile_pool(name="gacc", bufs=1, space="PSUM") as gaccp,
            tc.tile_pool(name="prod", bufs=PP_BUFS, space="PSUM") as prodp,
            tc.tile_pool(name="prodY", bufs=1, space="PSUM") as prodpY,
        ):
            # Weight tiles are declared here but loaded AFTER the first f
            # tile so HWDGE latency doesn't delay the pipeline start (the
            # first PE matmul needs xba only ~3us after f0 lands, by which
            # time the small weight DMAs have caught up).
            xbs = wpool.tile([PK, MS], bf16)
            sas = wpool.tile([PK, MS], bf16)
            xba = wpool.tile([128, MA], bf16)
            saa = wpool.tile([128, MA], bf16)
            ones = wpool.tile([128, 1], bf16)

            def load_weights():
                nc.sync.dma_start(xba[:], xba_d[:])
                nc.sync.dma_start(saa[:], saa_d[:])
                nc.sync.dma_start(ones[:], ones_d[:])
                nc.sync.dma_start(xbs[:], xbs_d[:])
                nc.sync.dma_start(sas[:], sas_d[:])

            # All accumulators share one PSUM bank; only the very first
            # gram matmul carries start=True (clears the bank); later
            # matmuls accumulate (or overwrite-onto-zeros where
            # has_written is still clear).
            gboth = gaccp.tile([128, GCOLS], f32)
            g1 = gboth[:, 0:128]
            g2 = gboth[:, 128:256]

            state = {"gfirst": True}

            def gram(field, ap_lo, width, acc):
                # accumulate lhsT.T@lhsT for col chunks of [ap_lo, ap_lo+width)
                c0 = 0
                while c0 < width:
                    cn = min(128, width - c0)
                    lhs = field[:, ap_lo + c0 : ap_lo + c0 + cn]
                    nc.tensor.matmul(
                        acc[0:cn, 0:cn], lhs, lhs,
                        start=state["gfirst"], stop=False,
                        skip_group_check=True,
                    )
                    state["gfirst"] = False
                    c0 += cn

            def ones_reduce(sqs, m, jn, scol):
                # Reduce an ACT-squared SBUF tile into PSUM accumulator
                # column `scol` via ones-vector matmuls (out free size 1:
                # ~1 PE cycle per 128 cols).
                for j in range(jn):
                    c0 = 0
                    while c0 < ZW:
                        cn = min(128, ZW - c0)
                        nc.tensor.matmul(
                            gboth[0:cn, scol : scol + 1],
                            sqs[0:m, j, c0 : c0 + cn], ones[0:m, :],
                            start=False, stop=False,
                            skip_group_check=True,
                        )
                        c0 += cn

            # Deferred PE work (grams / ones-reductions), with eligibility
            # aging so the in-order PE stream never waits on a producer:
            #   - grams become eligible a full iteration (or two, for the
            #     slower Pool/DMA-produced fields) after emission
            #   - ones-reductions become eligible 2 produce-chunks after
            #     their ACT square is issued
            ready = []            # [(cost, item)] eligible now
            ready_ns = [0.0]
            defer_map = {}        # iter idx -> [(cost, item)]
            ones_wait = []        # [(eligible_tick, cost, item)]
            tick = [0]

            def _pend_iter(it, item, cost):
                defer_map.setdefault(it, []).append((cost, item))

            def _pend_ones(item, cost):
                ones_wait.append((tick[0] + ONES_AGE, cost, item))

            def promote(it):
                for (cost, item) in defer_map.pop(it, []):
                    ready.append((cost, item))
                    ready_ns[0] += cost

            def _emit(item):
                if item[0] == "gram":
                    (_, t, k, acc) = item
                    gram(t[:, k, :], 0, ZW, acc)
                else:
                    (_, sqs, m, jn, scol) = item
                    ones_reduce(sqs, m, jn, scol)

            def filler(budget_ns):
                tick[0] += 1
                while ones_wait and ones_wait[0][0] <= tick[0]:
                    (_, cost, item) = ones_wait.pop(0)
                    _emit(item)
                    budget_ns -= cost
                while ready and budget_ns > 0:
                    cost, item = ready.pop(0)
                    ready_ns[0] -= cost
                    budget_ns -= cost
                    _emit(item)

            def flush_pending():
                for it in sorted(defer_map):
                    promote(it)
                for (_, cost, item) in ones_wait:
                    _emit(item)
                ones_wait.clear()
                for (cost, item) in ready:
                    _emit(item)
                ready.clear()
                ready_ns[0] = 0.0

            def produce_sq(w, rhs_fn, nblk, m, scol, fill_each):
                # PE stencil production of x-blocks into a multi-bank PSUM
                # tile (each block 512-padded = 1 bank), then ONE square
                # per chunk on ACT into an SBUF bf16 tile (no accum_out),
                # reduced later by PE ones-matmuls (queued as filler).
                for j0 in range(0, nblk, CHUNK):
                    jn = min(CHUNK, nblk - j0)
                    pp = prodp.tile([m, CHUNK, 512], f32, tag="pp")
                    for j in range(jn):
                        rhs = rhs_fn(j0 + j)
                        nc.tensor.matmul(
                            pp[:, j, 0 : rhs.shape[-1]], w, rhs,
                            start=True, stop=True, skip_group_check=True,
                        )
                    filler(fill_each)
                    sqs = sqpool.tile([m, CHUNK, ZW], bf16, tag="sqs")
                    nc.scalar.activation(
                        sqs[0:m, 0:jn, :], pp[0:m, 0:jn, 0:ZW], SQ,
                    )
                    _pend_ones(("ones", sqs, m, jn, scol), 40.0)

            def nchunks(spec):
                # PE/ACT chunk count for one group's yy+yz+xy stages
                nout = spec[4]
                return 3 * ((nout + CHUNK - 1) // CHUNK)

            prev_uts = []

            def body_pre(f, spec, gidx):
                # Emits the DVE/Pool/DMA production for one group: gz, gx,
                # u-path DMA shifts, xx, xz (DVE), zz (Pool). The PREVIOUS
                # group's u-path subs are emitted here first (their DMA
                # shift copies have had a full group to land, so the
                # in-order DVE stream never waits on DMA).
                (_, _, npart, nin, nout, _, _, m, is_main) = spec
                for (ga, gb, ugidx) in prev_uts:
                    xyu = uxy.tile([MA, 1, ZW], bf16, tag="xyu")
                    nc.vector.tensor_sub(xyu[:, 0, :], ga, gb)
                    _pend_iter(ugidx + 1, ("gram", xyu, 0, g2), 195.0)
                prev_uts.clear()
                gz = gzpool.tile([npart, nout, ZC - 6], bf16, tag="gz")
                nc.vector.tensor_sub(
                    gz[:], f[:, 2 : 2 + nout, 6:ZC], f[:, 2 : 2 + nout, 0 : ZC - 6]
                )
                gx = gxpool.tile([npart, nin - 2, ZC - 6], bf16, tag="gx")
                nc.vector.tensor_sub(
                    gx[:], f[:, 2:nin, 0 : ZC - 6], f[:, 0 : nin - 2, 0 : ZC - 6]
                )
                # u-path: selected main xy blocks via partition-shifted DMA
                # copies + DVE sub + gram (issued early for DMA latency)
                ublocks = []
                if is_main and U_OF:
                    ublocks = [k for k in range(nout)
                               if (gidx * XO + k) % U_MOD < U_OF
                               or ((gidx * XO + k) % U_EXTRA == 1
                                   if U_EXTRA else False)]
                    if gidx <= 2 and EARLY_U == 1:
                        ublocks = list(range(nout))
                    elif gidx <= 2 and EARLY_U == 0:
                        ublocks = []
                for k in ublocks:
                    ga = ushift.tile([MA, ZW], bf16, tag="ga")
                    nc.sync.dma_start(ga[:], gx[3 : 3 + MA, k + 1, 6 : 6 + ZW])
                    gb = ushift.tile([MA, ZW], bf16, tag="gb")
                    nc.sync.dma_start(gb[:], gx[1 : 1 + MA, k + 1, 6 : 6 + ZW])
                    prev_uts.append((ga[:], gb[:], gidx))
                xx = fldpool.tile([npart, nout, ZW], bf16, tag="xx")
                nc.vector.tensor_sub(
                    xx[:], gx[:, 2 : 2 + nout, 6 : ZC - 6],
                    gx[:, 0:nout, 6 : ZC - 6]
                )
                # xz: DVE produces the head blocks, Pool the tail (emitted
                # before zz so its grams, aged one less, are ready in time)
                xz = fldpool.tile([npart, nout, ZW], bf16, tag="xz")
                nxzp = nout // PZ_EVERY if (is_main and PZ_EVERY) else 0
                ndve = nout - nxzp
                nc.vector.tensor_sub(
                    xz[:, 0:ndve, :], gx[:, 1 : 1 + ndve, 6 : 6 + ZW],
                    gx[:, 1 : 1 + ndve, 0:ZW]
                )
                if nxzp:
                    nc.gpsimd.tensor_sub(
                        xz[:, ndve:nout, :],
                        gx[:, 1 + ndve : 1 + nout, 6 : 6 + ZW],
                        gx[:, 1 + ndve : 1 + nout, 0:ZW]
                    )
                # zz on Pool (GPSIMD), in sub-chunks so downstream grams
                # unblock progressively rather than after one huge op
                zz = fldpool.tile([npart, nout, ZW], bf16, tag="zz")
                for p0 in range(0, nout, PCHUNK):
                    pn = min(PCHUNK, nout - p0)
                    nc.gpsimd.tensor_sub(
                        zz[:, p0 : p0 + pn, :],
                        gz[:, p0 : p0 + pn, 6 : 6 + ZW],
                        gz[:, p0 : p0 + pn, 0:ZW],
                    )
                for k in range(nout):
                    _pend_iter(gidx, ("gram", xx, k, g1), 195.0)
                    _pend_iter(gidx, ("gram", xz, k, g2), 195.0)
                for k in range(nout):
                    _pend_iter(gidx + 1, ("gram", zz, k, g1), 195.0)
                return {"gz": gz, "gx": gx, "ublocks": ublocks}

            def produce_yy(f, spec, fill_each):
                (_, _, _, _, nout, _, _, m, _) = spec
                w_xb = spec[5][:]
                produce_sq(w_xb, lambda k: f[:, k + 2, 6 : 6 + ZW],
                           nout, m, 256, fill_each)

            def produce_yz(pre, spec, fill_each):
                (_, _, _, _, nout, _, _, m, _) = spec
                w_sa = spec[6][:]
                gz = pre["gz"]
                produce_sq(w_sa, lambda k: gz[:, k, 3 : 3 + ZW],
                           nout, m, 257, fill_each)

            def produce_xy(pre, spec, fill_each):
                (_, _, _, _, nout, _, _, m, _) = spec
                w_sa = spec[6][:]
                gx = pre["gx"]
                nonu = [k for k in range(nout) if k not in pre["ublocks"]]
                for j0 in range(0, len(nonu), CHUNK):
                    jn = min(CHUNK, len(nonu) - j0)
                    pp = prodp.tile([m, CHUNK, 512], f32, tag="pp")
                    for j in range(jn):
                        k = nonu[j0 + j]
                        nc.tensor.matmul(
                            pp[:, j, 0:ZW], w_sa, gx[:, k + 1, 6 : ZC - 6],
                            start=True, stop=True, skip_group_check=True,
                        )
                    filler(fill_each)
                    sqs = sqpool.tile([m, CHUNK, ZW], bf16, tag="sqs")
                    nc.scalar.activation(
                        sqs[0:m, 0:jn, :], pp[0:m, 0:jn, 0:ZW], SQ,
                    )
                    _pend_ones(("ones", sqs, m, jn, 257), 40.0)

            # group specs: (dram, start, npart, nin, nout, wxb, wsa, m, is_main)
            mspecs = [
                (fy_d, b0, 128, nin, nout, xba, saa, MA, True)
                for (b0, nin, nout) in groups
            ]
            sspecs = [
                (fs_d, s0, PK, nin, nout, xbs, sas, MS, False)
                for (s0, nin, nout) in subs
            ]
            specs = mspecs + sspecs

            def load(spec, split=0):
                d, o0, npart, nin = spec[0], spec[1], spec[2], spec[3]
                ft = fpool.tile([npart, nin, ZC], bf16, tag="fA")
                if split:
                    # split the critical first load so the first DVE sub
                    # (which reads only blocks [2, 2+nout)) starts sooner
                    nc.sync.dma_start(ft[:, 0:split, :],
                                      d[0:npart, o0 : o0 + split, :])
                    nc.sync.dma_start(ft[:, split:nin, :],
                                      d[0:npart, o0 + split : o0 + nin, :])
                else:
                    nc.sync.dma_start(ft[:], d[0:npart, o0 : o0 + nin, :])
                return ft

            # Software pipeline, one group of lookahead on the yy stage so
            # ACT crosses group boundaries without a bubble:
            #   iter i: yz(i), xy(i), body_pre(i+1), yy(i+1)
            # 2-ahead DMA prefetch (F_BUFS=3).
            fq = [load(specs[0])]
            load_weights()
            fq.append(load(specs[1]))
            pres = [body_pre(fq[0], specs[0], 0)]
            produce_yy(fq[0], specs[0], 400.0)
            for i, spec in enumerate(specs):
                if i + 2 < len(specs):
                    fq.append(load(specs[i + 2]))
                fcur = fq.pop(0)
                pre = pres.pop(0)
                promote(i)
                if i >= len(specs) - 2:
                    # pull the tail backlog forward so the final flush
                    # (pure-PE, ACT idle) stays short
                    promote(i + 1)
                    if i == len(specs) - 1:
                        promote(i + 2)
                # spread the eligible gram/ones backlog over this
                # iteration's PE chunks
                nck = nchunks(spec)
                fill_each = max(FILL_FLOOR, ready_ns[0] / max(1, nck))
                produce_yz(pre, spec, fill_each)
                produce_xy(pre, spec, fill_each)
                if i + 1 < len(specs):
                    pres.append(body_pre(fq[0], specs[i + 1], i + 1))
                    produce_yy(fq[0], specs[i + 1], fill_each)
            for (ga, gb, ugidx) in prev_uts:
                xyu = uxy.tile([MA, 1, ZW], bf16, tag="xyu")
                nc.vector.tensor_sub(xyu[:, 0, :], ga, gb)
                _pend_iter(ugidx + 1, ("gram", xyu, 0, g2), 195.0)
            prev_uts.clear()
            flush_pending()

            gc_ = accpool.tile([128, GCOLS], f32)
            nc.scalar.copy(gc_[:], gboth[:])
            nc.sync.dma_start(g_d[:], gc_[:])

    _split_multi_waits(nc, mybir)
    return nc, {}


def _host_inputs(ddf):
    """Per-core input dicts. ddf: (4, 160, 160, 160, 3) fp32 numpy."""
    xba, saa, xbs, sas = _stencil_mats()
    bf = ml_dtypes.bfloat16
    ones = np.ones((128, 1), bf)
    ddf = np.asarray(ddf, dtype=np.float32).astype(bf)
    in_maps = []
    for core in range(NCORES):
        b, xh = core // 2, core % 2
        x0 = 0 if xh == 0 else D - XIN
        # (x, y, z, c) -> (y, x, zc)
        vol = ddf[b, x0 : x0 + XIN]                       # (82, 160, 160, 3)
        fy = np.ascontiguousarray(
            vol.transpose(1, 0, 2, 3).reshape(D, XIN, ZC)
        )
        # packed strip: (3 xgrp x 36 y, 30 x-blocks, 480)
        fs = np.empty((PK, SIN, ZC), bf)
        for g in range(SGRP):
            xs = SOUT * g                                 # in-block start
            fs[g * YSN : (g + 1) * YSN] = fy[YS0:D, xs : xs + SIN, :]
        in_maps.append(
            {"fy": fy, "fs": fs, "xba": xba, "saa": saa, "xbs": xbs,
             "sas": sas, "ones": ones}
        )
    return in_maps


def _host_plane_correction(ddf):
    """Sum over junk y-planes J of (xx^2 + zz^2 + 2*xz^2) with the raw
    integer stencils, x,z windows [2,158). Returns (4,) fp64.
    Matches the device bf16 rounding of the input."""
    J = [0, 1, 124, 125, 126, 127, 158, 159]
    bf = ml_dtypes.bfloat16
    f = np.asarray(ddf, np.float32).astype(bf).astype(np.float64)
    f = f[:, :, J, :, :]                                  # (4, 160, 8, 160, 3)
    xx = f[:, 4:, :, :, :] - 2.0 * f[:, 2:-2, :, :, :] + f[:, :-4, :, :, :]
    zz = f[:, :, :, 4:, :] - 2.0 * f[:, :, :, 2:-2, :] + f[:, :, :, :-4, :]
    gx = f[:, 2:, :, :, :] - f[:, :-2, :, :, :]
    xz = gx[:, 1:-1, :, 2:, :] - gx[:, 1:-1, :, :-2, :]
    out = np.zeros(B)
    out += (xx[:, :, :, 2:-2, :] ** 2).sum(axis=(1, 2, 3, 4))
    out += (zz[:, 2:-2, :, :, :] ** 2).sum(axis=(1, 2, 3, 4))
    out += 2.0 * (xz[:, :, :, 1:-1, :] ** 2).sum(axis=(1, 2, 3, 4))
    return out


def kernel(ddf):
    ddf = np.asarray(ddf, dtype=np.float32)
    if "prog" not in _cache:
        _cache["prog"] = _build_program()
    nc, meta = _cache["prog"]

    from concourse.bass_utils import run_bass_kernel_spmd

    in_maps = _host_inputs(ddf)
    res = run_bass_kernel_spmd(nc, in_maps, list(range(NCORES)))

    corr = _host_plane_correction(ddf)
    out = np.zeros(B, np.float64)
    for core in range(NCORES):
        r = res.results[core]
        b = core // 2
        g = r["g"].astype(np.float64)
        out[b] += np.trace(g[:, 0:128])
        out[b] += 2.0 * np.trace(g[:, 128:256])
        out[b] += g[:, 256].sum()
        out[b] += 2.0 * g[:, 257].sum()
    out -= corr
    out /= 16.0 * (156 ** 3) * 3
    return out.astype(np.float32)
